# revision 35
# baseline (speedup 1.0000x reference)
"""GAT (2-layer, DGL-style) Bass kernel for 8 Trainium2 NeuronCores.

Contract: kernel(**inputs) takes FULL unsharded inputs (as produced by the
problem's setup_inputs) and returns the FULL [N, NCLS] float32 output.

Sharding: nodes are dst-sharded across the 8 cores (core c owns nodes
[c*S, (c+1)*S)); each core processes the edges whose dst lands in its shard,
sorted by dst and packed into 128-slot destination windows.  The layer-1
feature table is computed replicated (full x @ W1ext per core) which avoids a
large slow collective; only the small layer-2 table (66 cols/node) is
exchanged with a single AllGather.

Per edge-window the kernel does batched indirect-DMA row gathers, computes
edge softmax numerators p = exp(leaky_relu(el[src]+er[dst])) (as
max(exp(e), exp(0.2e))), and performs the segment-sum scatter as one-hot
matmuls accumulating in PSUM; appending p itself as an extra rhs column
yields the softmax denominator in the same matmuls.
"""

import math
from contextlib import ExitStack
from dataclasses import dataclass

import numpy as np

from concourse import bacc, bass, mybir, tile
from concourse.bass import IndirectOffsetOnAxis
from concourse.bass_utils import run_bass_kernel_spmd
from concourse.masks import make_identity

f32 = mybir.dt.float32
i32 = mybir.dt.int32
ALU = mybir.AluOpType
ACTF = mybir.ActivationFunctionType

P = 128


@dataclass
class Cfg:
    N: int = 100000
    E: int = 1600000
    IN: int = 256
    HID: int = 128
    HEADS: int = 8
    DH: int = 16
    NCLS: int = 64
    C: int = 8  # cores
    SLOPE: float = 0.2

    @property
    def S(self):  # nodes per shard
        assert self.N % self.C == 0
        return self.N // self.C

    @property
    def F1(self):  # table1 row: h(HID) | el(HEADS) | er(HEADS)
        return self.HID + 2 * self.HEADS

    @property
    def G1(self):  # gathered cols for layer-1 edges: h | el
        return self.HID + self.HEADS

    @property
    def F2(self):  # table2 row: h2(NCLS) | el2(1) | er2(1)
        return self.NCLS + 2

    @property
    def G2(self):
        return self.NCLS + 1

    @property
    def NW(self):  # dst windows per shard
        return (self.S + P - 1) // P


def host_prep(cfg: Cfg, src: np.ndarray, dst: np.ndarray):
    """Sort/partition edges by (dst shard, dst window); pack indices
    partition-major into [128, T] arrays with a tile schedule common to all
    cores (required: one SPMD program)."""
    S, NW, C = cfg.S, cfg.NW, cfg.C
    shard = dst // S
    per_core = []
    cnts = np.zeros((C, NW), dtype=np.int64)
    for c in range(C):
        m = shard == c
        s_c = src[m].astype(np.int64)
        d_c = dst[m].astype(np.int64)
        dl = d_c - c * S
        order = np.argsort(dl, kind="stable")
        s_c, d_c, dl = s_c[order], d_c[order], dl[order]
        w = dl >> 7
        cnts[c] = np.bincount(w, minlength=NW)
        per_core.append((s_c, d_c, dl))
    kw = np.maximum(1, np.ceil(cnts.max(axis=0) / P)).astype(np.int64)
    offs = np.zeros(NW + 1, dtype=np.int64)
    offs[1:] = np.cumsum(kw)
    T = int(offs[-1])

    packs = []
    for c in range(C):
        s_c, d_c, dl = per_core[c]
        src_pack = np.zeros((P, T), np.int32)
        dstl_pack = np.full((P, T), -1.0, np.float32)
        pos = 0
        for w in range(NW):
            n = int(cnts[c, w])
            k = int(kw[w])
            sb = np.zeros(k * P, np.int64)
            lb = np.full(k * P, -1.0, np.float32)
            sb[:n] = s_c[pos : pos + n]
            lb[:n] = (dl[pos : pos + n] - w * P).astype(np.float32)
            o = offs[w]
            src_pack[:, o : o + k] = sb.reshape(k, P).T
            dstl_pack[:, o : o + k] = lb.reshape(k, P).T
            pos += n
        dstlT_pack = np.ascontiguousarray(dstl_pack.T)
        packs.append((src_pack, dstl_pack, dstlT_pack))
    return kw, offs, T, packs


def _ap(base: bass.AP, extra_offset_elems: int, dims):
    """Clone an AP with a custom [step, count] dim list (partition dim kept)."""
    return bass.AP(
        tensor=base.tensor,
        offset=base.offset + extra_offset_elems,
        ap=[list(base.ap[0])] + [list(d) for d in dims],
    )


def build_program(cfg: Cfg, kw, offs, T, debug_stage: int = 0, repeat: int = 1):
    """debug_stage: 0=full kernel; 1=stop after stage A (out=table1);
    2=stop after edge phase 1 (out=h2sh).  repeat: emit the body N times
    (idempotent) so wall(N)-wall(1) isolates device exec time."""
    nc = bacc.Bacc(
        "TRN2",
        target_bir_lowering=False,
        debug=False,
        enable_asserts=False,
        num_devices=cfg.C,
    )
    S, NW = cfg.S, cfg.NW
    IN, HID, HEADS, NCLS = cfg.IN, cfg.HID, cfg.HEADS, cfg.NCLS
    F1, G1, F2, G2 = cfg.F1, cfg.G1, cfg.F2, cfg.G2
    KC = (IN + P - 1) // P  # input-dim chunks

    # ---- I/O ----
    xT_d = nc.dram_tensor("xT", [IN, cfg.N], f32, kind="ExternalInput").ap()
    W1_d = nc.dram_tensor("W1", [IN, HID], f32, kind="ExternalInput").ap()
    W1T_d = nc.dram_tensor("W1T", [HID, IN], f32, kind="ExternalInput").ap()
    alar1_d = nc.dram_tensor("alar1", [HID, 2 * HEADS], f32, kind="ExternalInput").ap()
    b1_d = nc.dram_tensor("b1", [HID], f32, kind="ExternalInput").ap()
    W2_d = nc.dram_tensor("W2", [HID, NCLS], f32, kind="ExternalInput").ap()
    W2T_d = nc.dram_tensor("W2T", [NCLS, HID], f32, kind="ExternalInput").ap()
    alar2_d = nc.dram_tensor("alar2", [NCLS, 2], f32, kind="ExternalInput").ap()
    b2_d = nc.dram_tensor("b2", [NCLS], f32, kind="ExternalInput").ap()
    srcp_d = nc.dram_tensor("src_pack", [P, T], i32, kind="ExternalInput").ap()
    dstl_d = nc.dram_tensor("dstl_pack", [P, T], f32, kind="ExternalInput").ap()
    dstlT_d = nc.dram_tensor("dstlT_pack", [T, P], f32, kind="ExternalInput").ap()
    if debug_stage == 0:
        out_d = nc.dram_tensor("out", [S, NCLS], f32, kind="ExternalOutput").ap()

    # ---- internal DRAM ----
    # table rows: [h | el | er]; gathers read full rows (er cols unused there),
    # er is read per dst-window as a sequential strided slice.
    W1ext_d = nc.dram_tensor("W1ext", [IN, F1], f32).ap()
    if debug_stage == 1:
        table1_d = nc.dram_tensor("out", [cfg.N, F1], f32, kind="ExternalOutput").ap()
    else:
        table1_d = nc.dram_tensor("table1", [cfg.N, F1], f32).ap()
    if debug_stage == 2:
        h2sh_d = nc.dram_tensor("out", [S, F2], f32, kind="ExternalOutput").ap()
    else:
        h2sh_d = nc.dram_tensor("h2sh", [S, F2], f32).ap()
    h2full_d = nc.dram_tensor("h2full", [cfg.N, F2], f32, addr_space="Shared").ap()

    with tile.TileContext(nc) as tc, ExitStack() as octx:
        const = octx.enter_context(tc.tile_pool(name="const", bufs=1))

        # ---- constants ----
        identity = const.tile([P, P], f32)
        make_identity(nc, identity[:])
        iota_i = const.tile([P, P], i32)
        nc.gpsimd.iota(iota_i[:], pattern=[[1, P]], base=0, channel_multiplier=0)
        iota_f = const.tile([P, P], f32)
        nc.vector.tensor_copy(iota_f[:], iota_i[:])
        iota_ci = const.tile([P, 1], i32)
        nc.gpsimd.iota(iota_ci[:], pattern=[[0, 1]], base=0, channel_multiplier=1)
        iota_c = const.tile([P, 1], f32)
        nc.vector.tensor_copy(iota_c[:], iota_ci[:])
        ones_row = const.tile([1, P], f32)
        nc.vector.memset(ones_row[:], 1.0)

        with ExitStack() as ictx:
            ipool = ictx.enter_context(tc.tile_pool(name="init_sb", bufs=2))
            ipsum = ictx.enter_context(
                tc.tile_pool(name="init_ps", bufs=2, space="PSUM")
            )

            # bias matrices (b broadcast down partitions via ones-row matmul)
            b1row = ipool.tile([1, HID], f32)
            nc.sync.dma_start(out=b1row[:], in_=b1_d[:].rearrange("(a b) -> a b", a=1))
            b2row = ipool.tile([1, NCLS], f32)
            nc.sync.dma_start(out=b2row[:], in_=b2_d[:].rearrange("(a b) -> a b", a=1))
            bias1 = const.tile([P, HID], f32)
            pb1 = ipsum.tile([P, HID], f32)
            nc.tensor.matmul(pb1[:], lhsT=ones_row[:], rhs=b1row[:], start=True, stop=True)
            nc.vector.tensor_copy(bias1[:], pb1[:])
            bias2 = const.tile([P, NCLS], f32)
            pb2 = ipsum.tile([P, NCLS], f32)
            nc.tensor.matmul(pb2[:], lhsT=ones_row[:], rhs=b2row[:], start=True, stop=True)
            nc.vector.tensor_copy(bias2[:], pb2[:])

            # W2ext = [W2 | W2@al2 | W2@ar2]   [HID, F2]
            W2ext = const.tile([P, F2], f32)
            nc.sync.dma_start(out=W2ext[:HID, 0:NCLS], in_=W2_d[:, :])
            w2t_sb = ipool.tile([NCLS, HID], f32)
            nc.sync.dma_start(out=w2t_sb[:], in_=W2T_d[:, :])
            alar2_sb = ipool.tile([NCLS, 2], f32)
            nc.sync.dma_start(out=alar2_sb[:], in_=alar2_d[:, :])
            pw2 = ipsum.tile([P, 2], f32)
            nc.tensor.matmul(pw2[:HID, :], lhsT=w2t_sb[:NCLS, :], rhs=alar2_sb[:NCLS, :], start=True, stop=True)
            nc.vector.tensor_copy(W2ext[:HID, NCLS:F2], pw2[:HID, :])

            # W1ext = [W1 | W1@alar1]   [IN, F1]  (assembled in DRAM)
            alar1_sb = ipool.tile([HID, 2 * HEADS], f32)
            nc.sync.dma_start(out=alar1_sb[:], in_=alar1_d[:, :])
            for kc in range(KC):
                cw = min(P, IN - kc * P)
                w1sb = ipool.tile([P, HID], f32, tag="w1sb")
                nc.sync.dma_start(out=w1sb[:cw, :], in_=W1_d[kc * P : kc * P + cw, :])
                nc.sync.dma_start(out=W1ext_d[kc * P : kc * P + cw, 0:HID], in_=w1sb[:cw, :])
                w1t_sb = ipool.tile([HID, P], f32, tag="w1t")
                nc.sync.dma_start(out=w1t_sb[:, :cw], in_=W1T_d[:, kc * P : kc * P + cw])
                pwe = ipsum.tile([P, 2 * HEADS], f32, tag="pwe")
                nc.tensor.matmul(pwe[:cw, :], lhsT=w1t_sb[:HID, :cw], rhs=alar1_sb[:HID, :], start=True, stop=True)
                wext_sb = ipool.tile([P, 2 * HEADS], f32, tag="wext")
                nc.vector.tensor_copy(wext_sb[:cw, :], pwe[:cw, :])
                nc.sync.dma_start(out=W1ext_d[kc * P : kc * P + cw, HID:F1], in_=wext_sb[:cw, :])

        # ---- stage A: table1 = x @ W1ext   (replicated full N) ----
        def stage_a(actx: ExitStack):
            w1e_pool = actx.enter_context(tc.tile_pool(name="a_w1e", bufs=1))
            w1e = []
            for kc in range(KC):
                cw = min(P, IN - kc * P)
                t = w1e_pool.tile([P, F1], f32, tag=f"w1e{kc}")
                nc.sync.dma_start(out=t[:cw, :], in_=W1ext_d[kc * P : kc * P + cw, :])
                w1e.append((t, cw))
            ax_pool = actx.enter_context(tc.tile_pool(name="a_x", bufs=3))
            ast_pool = actx.enter_context(tc.tile_pool(name="a_stage", bufs=3))
            aps_pool = actx.enter_context(tc.tile_pool(name="a_ps", bufs=3, space="PSUM"))
            GA = 512
            for g0 in range(0, cfg.N, GA):
                gw = min(GA, cfg.N - g0)
                xa = ax_pool.tile([P, KC * gw], f32, tag="xa")
                for kc in range(KC):
                    cw = min(P, IN - kc * P)
                    nc.sync.dma_start(
                        out=xa[:cw, kc * gw : kc * gw + gw],
                        in_=xT_d[kc * P : kc * P + cw, g0 : g0 + gw],
                    )
                for t0 in range(0, gw, P):
                    tw = min(P, gw - t0)
                    ps = aps_pool.tile([P, F1], f32, tag="aps")
                    for kc in range(KC):
                        cw = min(P, IN - kc * P)
                        nc.tensor.matmul(
                            ps[:tw, :],
                            lhsT=xa[:cw, kc * gw + t0 : kc * gw + t0 + tw],
                            rhs=w1e[kc][0][:cw, :],
                            start=(kc == 0),
                            stop=(kc == KC - 1),
                        )
                    st = ast_pool.tile([P, F1], f32, tag="ast")
                    nc.vector.tensor_copy(st[:tw, :], ps[:tw, :])
                    nc.sync.dma_start(
                        out=table1_d[g0 + t0 : g0 + t0 + tw, :], in_=st[:tw, :]
                    )

        # ---- edge phases ----
        def edge_phase(layer: int, ectx: ExitStack):
            if layer == 1:
                TBL, GW, NH = table1_d, F1, HEADS
                # er rows for own shard live in the global (replicated) table
                # at runtime offset core_id*S
                sbase = nc.partition_id() * S
            else:
                TBL, GW, NH = h2full_d, F2, 1
            MW = GW - NH      # matmul width: [h | p] cols
            elc = MW - NH     # el column offset within a row
            ip = ectx.enter_context(tc.tile_pool(name=f"e{layer}_idx", bufs=4))
            gp = ectx.enter_context(tc.tile_pool(name=f"e{layer}_g", bufs=4))
            op = ectx.enter_context(tc.tile_pool(name=f"e{layer}_oh", bufs=3))
            wp = ectx.enter_context(tc.tile_pool(name=f"e{layer}_w", bufs=4))
            sp = ectx.enter_context(tc.tile_pool(name=f"e{layer}_s", bufs=4))
            pp = ectx.enter_context(tc.tile_pool(name=f"e{layer}_ps", bufs=3, space="PSUM"))
            # dx and pse share one pool/tag (sequential lifetimes per window)
            pep = ectx.enter_context(tc.tile_pool(name=f"e{layer}_pse", bufs=2, space="PSUM"))
            pxp = pep
            if layer == 1:
                ptp = ectx.enter_context(tc.tile_pool(name="e1_pst", bufs=2, space="PSUM"))

            for w in range(NW):
                k = int(kw[w])
                o = int(offs[w])
                base = w * P
                ns = min(P, S - base)

                src_t = ip.tile([P, k], i32, tag="src")
                dstl_t = ip.tile([P, k], f32, tag="dstl")
                dstlT_t = ip.tile([1, k * P], f32, tag="dstlT")
                nc.sync.dma_start(out=src_t[:], in_=srcp_d[:, o : o + k])
                nc.sync.dma_start(out=dstl_t[:], in_=dstl_d[:, o : o + k])
                nc.sync.dma_start(
                    out=dstlT_t[:],
                    in_=bass.AP(
                        tensor=dstlT_d.tensor, offset=o * P, ap=[[1, 1], [1, k * P]]
                    ),
                )

                # walrus honors ONE offset per partition per indirect DMA, so
                # gather 128 rows (one edge-tile) per instruction.
                G = gp.tile([P, k * GW], f32, tag="G")
                for j in range(k):
                    nc.gpsimd.indirect_dma_start(
                        out=G[:, j * GW : (j + 1) * GW],
                        out_offset=None,
                        in_=TBL[:, :],
                        in_offset=IndirectOffsetOnAxis(ap=src_t[:, j : j + 1], axis=0),
                    )

                # er for this window's 128 dst slots (sequential strided read)
                erwin = sp.tile([P, NH], f32, tag="erwin")
                nc.vector.memset(erwin[:], 0.0)
                if layer == 1:
                    er_src = TBL[bass.ds(sbase + base, ns), MW : MW + NH]
                else:
                    er_src = h2sh_d[base : base + ns, MW : MW + NH]
                nc.sync.dma_start(out=erwin[:ns, :], in_=er_src)

                # transposed one-hot  OHT[slot, edge] = (dstl[edge] == slot):
                # broadcast dstlT down partitions via ones-matmul, compare to
                # the per-partition iota column.
                OHT = op.tile([P, k * P], f32, tag="OHT")
                for c0 in range(0, k * P, 512):
                    cw = min(512, k * P - c0)
                    dx = pxp.tile([P, 512], f32, tag="dxpse")
                    nc.tensor.matmul(
                        dx[:, :cw], lhsT=ones_row[:], rhs=dstlT_t[:, c0 : c0 + cw],
                        start=True, stop=True,
                    )
                    nc.vector.tensor_scalar(
                        out=OHT[:, c0 : c0 + cw], in0=dx[:, :cw],
                        scalar1=iota_c[:, 0:1], scalar2=None, op0=ALU.is_equal,
                    )

                # er[dst] per edge via PE: er_exp = OHT.T @ erwin
                pse_full = pep.tile([P, 512], f32, tag="dxpse")
                pse = pse_full[:, : k * NH]
                for j in range(k):
                    nc.tensor.matmul(
                        pse[:, j * NH : (j + 1) * NH],
                        lhsT=OHT[:, j * P : (j + 1) * P],
                        rhs=erwin[:],
                        start=True, stop=True,
                    )

                # e = el[src] + er[dst]   [P, k*NH]
                el_view = _ap(G[:], elc, [[GW, k], [1, NH]])
                et = sp.tile([P, k * NH], f32, tag="et")
                nc.vector.tensor_tensor(out=et[:], in0=el_view, in1=pse[:], op=ALU.add)
                # p = exp(leaky_relu(e)) = max(exp(e), exp(slope*e)) -> G's el cols
                ea = sp.tile([P, k * NH], f32, tag="ea")
                nc.scalar.activation(ea[:], et[:], ACTF.Exp)
                eb = sp.tile([P, k * NH], f32, tag="eb")
                nc.scalar.activation(eb[:], et[:], ACTF.Exp, scale=cfg.SLOPE)
                nc.vector.tensor_tensor(out=el_view, in0=ea[:], in1=eb[:], op=ALU.max)

                # msg = p * h   (in place on G's h cols)
                if layer == 1:
                    h_view = _ap(G[:], 0, [[GW, k], [cfg.DH, HEADS], [1, cfg.DH]])
                    p_view = _ap(G[:], elc, [[GW, k], [1, HEADS], [0, cfg.DH]])
                else:
                    h_view = _ap(G[:], 0, [[GW, k], [1, NCLS]])
                    p_view = _ap(G[:], elc, [[GW, k], [0, NCLS]])
                nc.vector.tensor_tensor(out=h_view, in0=h_view, in1=p_view, op=ALU.mult)

                # one-hot dst-slot matrix  [P, k*P]
                OH = op.tile([P, k * P], f32, tag="OH")
                nc.vector.tensor_tensor(
                    out=_ap(OH[:], 0, [[P, k], [1, P]]),
                    in0=_ap(iota_f[:], 0, [[0, k], [1, P]]),
                    in1=_ap(dstl_t[:], 0, [[1, k], [0, P]]),
                    op=ALU.is_equal,
                )

                # scatter: psum[slot, :] = sum_e OH[e, slot] * Msg[e, :]
                ps = pp.tile([P, MW], f32, tag="eps")
                for j in range(k):
                    nc.tensor.matmul(
                        ps[:, :],
                        lhsT=OH[:, j * P : (j + 1) * P],
                        rhs=G[:, j * GW : j * GW + MW],
                        start=(j == 0),
                        stop=(j == k - 1),
                    )

                # normalize + bias
                scl = sp.tile([P, NH], f32, tag="scl")
                nc.vector.tensor_scalar(
                    out=scl[:], in0=ps[:, MW - NH : MW], scalar1=1e-30, scalar2=None, op0=ALU.max
                )
                rs = sp.tile([P, NH], f32, tag="rs")
                nc.vector.reciprocal(rs[:], scl[:])

                if layer == 1:
                    h1 = wp.tile([P, HID], f32, tag="h1")
                    nc.vector.tensor_tensor(
                        out=_ap(h1[:], 0, [[cfg.DH, HEADS], [1, cfg.DH]]),
                        in0=_ap(ps[:], 0, [[cfg.DH, HEADS], [1, cfg.DH]]),
                        in1=_ap(rs[:], 0, [[1, HEADS], [0, cfg.DH]]),
                        op=ALU.mult,
                    )
                    nc.vector.tensor_tensor(out=h1[:], in0=h1[:], in1=bias1[:], op=ALU.add)
                    # elu twice
                    cur = h1
                    for r in range(2):
                        tmin = wp.tile([P, HID], f32, tag=f"tmin{r}")
                        nc.vector.tensor_scalar(
                            out=tmin[:], in0=cur[:], scalar1=0.0, scalar2=None, op0=ALU.min
                        )
                        nc.scalar.activation(tmin[:], tmin[:], ACTF.Exp)
                        nc.vector.tensor_scalar(
                            out=tmin[:], in0=tmin[:], scalar1=-1.0, scalar2=None, op0=ALU.add
                        )
                        nxt = wp.tile([P, HID], f32, tag=f"helu{r}")
                        nc.vector.tensor_tensor(out=nxt[:], in0=cur[:], in1=tmin[:], op=ALU.max)
                        cur = nxt
                    # h2 = cur @ W2ext  (transpose then matmul)
                    pt = ptp.tile([P, P], f32, tag="pt")
                    nc.tensor.transpose(pt[:], cur[:], identity[:])
                    h1T = wp.tile([P, P], f32, tag="h1T")
                    nc.vector.tensor_copy(h1T[:], pt[:])
                    ps2 = ptp.tile([P, P], f32, tag="pt")
                    nc.tensor.matmul(
                        ps2[:, :F2], lhsT=h1T[:HID, :], rhs=W2ext[:HID, :],
                        start=True, stop=True,
                    )
                    h2w = wp.tile([P, F2], f32, tag="h2w")
                    nc.vector.tensor_copy(h2w[:], ps2[:, :F2])
                    nc.sync.dma_start(out=h2sh_d[base : base + ns, :], in_=h2w[:ns, :])
                else:
                    o2 = wp.tile([P, NCLS], f32, tag="o2")
                    nc.vector.tensor_scalar(
                        out=o2[:], in0=ps[:, 0:NCLS], scalar1=rs[:, 0:1], scalar2=None, op0=ALU.mult
                    )
                    nc.vector.tensor_tensor(out=o2[:], in0=o2[:], in1=bias2[:], op=ALU.add)
                    nc.sync.dma_start(out=out_d[base : base + ns, :], in_=o2[:ns, :])

        for _rep in range(repeat):
            with ExitStack() as actx:
                stage_a(actx)

            if debug_stage != 1:
                with ExitStack() as e1ctx:
                    edge_phase(1, e1ctx)

            if debug_stage == 0:
                nc.gpsimd.collective_compute(
                    "AllGather", ALU.bypass, replica_groups=[list(range(cfg.C))],
                    ins=[h2sh_d[:, :]], outs=[h2full_d[:, :]],
                )

                with ExitStack() as e2ctx:
                    edge_phase(2, e2ctx)

    nc.compile()
    return nc


def run(cfg: Cfg, inputs: dict, trace: bool = False, debug_stage: int = 0):
    x = np.asarray(inputs["x"], np.float32)
    src = np.asarray(inputs["src"])
    dst = np.asarray(inputs["dst"])
    W1 = np.asarray(inputs["W1"], np.float32)
    al1 = np.asarray(inputs["al1"], np.float32)
    ar1 = np.asarray(inputs["ar1"], np.float32)
    b1 = np.asarray(inputs["b1"], np.float32)
    W2 = np.asarray(inputs["W2"], np.float32)
    al2 = np.asarray(inputs["al2"], np.float32)
    ar2 = np.asarray(inputs["ar2"], np.float32)
    b2 = np.asarray(inputs["b2"], np.float32)

    kw, offs, T, packs = host_prep(cfg, np.asarray(src), np.asarray(dst))

    xT = np.ascontiguousarray(x.T)
    W1T = np.ascontiguousarray(W1.T)
    W2T = np.ascontiguousarray(W2.T)
    # alar1[16h+d, t]: t==h -> al1[h,d]; t==HEADS+h -> ar1[h,d]  (zero elsewhere)
    alar1 = np.zeros((cfg.HID, 2 * cfg.HEADS), np.float32)
    for h in range(cfg.HEADS):
        alar1[h * cfg.DH : (h + 1) * cfg.DH, h] = al1[h]
        alar1[h * cfg.DH : (h + 1) * cfg.DH, cfg.HEADS + h] = ar1[h]
    alar2 = np.stack([al2[0], ar2[0]], axis=1).astype(np.float32)

    nc = build_program(cfg, kw, offs, T, debug_stage=debug_stage)

    in_maps = []
    for c in range(cfg.C):
        sp_, lp_, lpT_ = packs[c]
        in_maps.append(
            {
                "xT": xT,
                "W1": W1,
                "W1T": W1T,
                "alar1": alar1,
                "b1": b1,
                "W2": W2,
                "W2T": W2T,
                "alar2": alar2,
                "b2": b2,
                "src_pack": sp_,
                "dstl_pack": lp_,
                "dstlT_pack": lpT_,
            }
        )

    res = run_bass_kernel_spmd(nc, in_maps, core_ids=list(range(cfg.C)), trace=trace)
    if debug_stage == 1:
        return [res.results[c]["out"] for c in range(cfg.C)], res
    out = np.concatenate([res.results[c]["out"] for c in range(cfg.C)], axis=0)
    return out, res


def kernel(**inputs) -> np.ndarray:
    cfg = Cfg()
    out, _ = run(cfg, inputs)
    return out.astype(np.float32)
